# revision 9
# baseline (speedup 1.0000x reference)
"""T5-style encoder self-attention (dense_transformer) on 8 Trainium2 NeuronCores.

Problem (full shapes): hidden [2,2048,2048], Wq/Wk/Wv/Wo [2048,2048],
rel_emb [32,32] (bidirectional T5 relative-position bias), mask [2,1,1,2048].

Sharding: data-parallel over batch (2) x tensor-parallel over heads (4 groups
of 8 heads) = 8 cores, Megatron-style. Each core computes a partial output
[2048,2048] for its batch (its 8 heads through its Wo row-slice); the host
sums 4 partials per batch.

Per-core kernel design (all matmuls at full PE rate):
  - projections in float32r (TF32-like, 1 cyc/row at N>=512):
      Q^T,K^T [hd,s] layouts (hd on partitions) direct from lhsT=W, rhs=x^T;
      V [s,hd] from lhsT=x^T-slices, rhs=Wv. x^T supplied by host (layout prep).
  - Q^T is stored with s REVERSED so the relative-position bias becomes a
    positive-shear Toeplitz: U_h[p,j] = exp(bias_h)[diag = p+j-2047], built
    with one contiguous sheared DMA per head.
  - scores^T tiles [k=128part, q=512free] = row-packed pair of K=64 f32r
    matmuls (tile_position (0,0)/(64,0)) for 2 heads concurrently.
  - softmax without max-subtraction (scores are O(1) by construction):
    ACT computes exp(s/8 + mask_k) psum->sbuf bf16; DVE multiplies by the
    Toeplitz exp-bias tile (bf16 2x mode).
  - PV with V_aug=[V | 1] (M=65): psum row 64 accumulates the softmax
    denominator for free; DVE normalizes while copying ctx^T (un-reversing q).
  - output projection: lhsT=ctx^T tiles, rhs=Wo rows, bf16, fp32 psum.

The relative-position bucket table is a host-side STRUCTURAL constant
(depends only on S, not on data); rel_emb values are gathered on device via a
one-hot matmul + exp.
"""

import math
import os
import sys

for _p in ("/opt/trn_rl_repo",):
    if _p not in sys.path:
        sys.path.insert(0, _p)

import numpy as np

import concourse.bass as bass
import concourse.mybir as mybir
import concourse.tile as tile
from concourse import bacc
from concourse.bass_utils import run_bass_kernel_spmd

DT = mybir.dt
AF = mybir.ActivationFunctionType
OP = mybir.AluOpType

# ---- problem constants (hardcoded per contract) ----
B, S, D = 2, 2048, 2048
N_HEADS, D_KV = 32, 64
NUM_BUCKETS, MAX_DISTANCE = 32, 128
NCORES = 8
HL = 8            # heads per core
P = 128
SC = 512          # free-dim chunk
NKT = S // P      # 16 k-tiles
NQC = S // SC     # 4 q-chunks
NDT = D // P      # 16 D-tiles
NMT = (HL * D_KV) // P   # 4 hd m-tiles per core
W_U = 3968        # toeplitz tile width: SC + (NKT-1)*P + ... = 512 + 1920*? -> k0+jg0 max 3456, +512
NDIAG = 4096      # ediag row stride (4095 used)


def _rel_bucket_host(d):
    """Exact numpy replica of reference._relative_position_bucket (fp32 math,
    int32 truncation) for bidirectional buckets. d = k - q (int array)."""
    num_buckets = NUM_BUCKETS // 2          # 16
    max_exact = num_buckets // 2            # 8
    rel = np.asarray(d, dtype=np.int64)
    buckets = (rel > 0).astype(np.int32) * num_buckets
    arel = np.abs(rel)
    is_small = arel < max_exact
    rp_safe = np.maximum(arel, 1).astype(np.float32)
    log_ratio = np.log(rp_safe / np.float32(max_exact)).astype(np.float32)
    scale = np.float32(math.log(MAX_DISTANCE / max_exact))
    rp_large = max_exact + (log_ratio / scale * np.float32(num_buckets - max_exact)).astype(np.int32)
    rp_large = np.minimum(rp_large, num_buckets - 1)
    buckets = buckets + np.where(is_small, arel.astype(np.int32), rp_large)
    return buckets.astype(np.int32)


def _onehot_const():
    """OH[u, i] = 1 if bucket(i - 2047) == u, i in [0, 4095); col 4095 = 0."""
    i = np.arange(NDIAG - 1)
    b = _rel_bucket_host(i - (S - 1))
    oh = np.zeros((NUM_BUCKETS, NDIAG), dtype=np.float32)
    oh[b, i] = 1.0
    return oh


def _build():
    nc = bacc.Bacc(None, name="attn_tp")

    xt = nc.declare_dram_parameter("xt", [D, S], DT.bfloat16, isOutput=False)
    wq = nc.declare_dram_parameter("wq", [D, HL * D_KV], DT.bfloat16, isOutput=False)
    wk = nc.declare_dram_parameter("wk", [D, HL * D_KV], DT.bfloat16, isOutput=False)
    wv = nc.declare_dram_parameter("wv", [D, HL * D_KV], DT.bfloat16, isOutput=False)
    wo = nc.declare_dram_parameter("wo", [HL * D_KV, D], DT.bfloat16, isOutput=False)
    mask = nc.declare_dram_parameter("mask", [S], DT.float32, isOutput=False)
    rel = nc.declare_dram_parameter("rel", [NUM_BUCKETS, HL], DT.float32, isOutput=False)
    oh = nc.declare_dram_parameter("oh", [NUM_BUCKETS, NDIAG], DT.float32, isOutput=False)
    out = nc.declare_dram_parameter("out", [S, D], DT.float32, isOutput=True)

    with tile.TileContext(nc) as tc:
        with (
            tc.tile_pool(name="res", bufs=1) as res,          # persistent tensors
            tc.tile_pool(name="xtp", bufs=3) as xtp,          # x^T stream tiles
            tc.tile_pool(name="stage", bufs=2) as stage,      # fp32 staging
            tc.tile_pool(name="upool", bufs=2) as upool,      # toeplitz exp-bias tiles
            tc.tile_pool(name="pexp", bufs=6) as pexpp,       # probs tiles
            tc.tile_pool(name="outp", bufs=3) as outp,        # out staging
            tc.tile_pool(name="ps", bufs=8, space="PSUM") as ps,
            tc.tile_pool(name="dram", bufs=1, space="DRAM") as dramp,
        ):
            # ---------- phase 0: constants / ediag ----------
            mask_sb = res.tile([P, NKT], DT.float32, tag="mask")
            nc.sync.dma_start(mask_sb[:], mask.ap().rearrange("(kt p) -> p kt", p=P))

            rel_sb = res.tile([NUM_BUCKETS, HL], DT.float32, tag="rel")
            nc.sync.dma_start(rel_sb[:], rel[:])

            ediag_dram = dramp.tile([HL, NDIAG], DT.bfloat16)
            ediag_sb = res.tile([HL, NDIAG], DT.bfloat16, tag="ediag")
            for c in range(NDIAG // SC):
                oh_sb = stage.tile([NUM_BUCKETS, SC], DT.float32, tag="oh")
                nc.sync.dma_start(oh_sb[:], oh[:, c * SC:(c + 1) * SC])
                ed_ps = ps.tile([HL, SC], DT.float32, tag="ps")
                nc.tensor.matmul(ed_ps[:], rel_sb[:], oh_sb[:], start=True, stop=True)
                nc.scalar.activation(
                    out=ediag_sb[:, c * SC:(c + 1) * SC], in_=ed_ps[:], func=AF.Exp
                )
            nc.sync.dma_start(ediag_dram[:], ediag_sb[:])

            # weights (resident, f32r)
            wq_sb = res.tile([P, NDT, HL * D_KV], DT.bfloat16, tag="wq")
            wk_sb = res.tile([P, NDT, HL * D_KV], DT.bfloat16, tag="wk")
            wv_sb = res.tile([P, NDT, HL * D_KV], DT.bfloat16, tag="wv")
            nc.sync.dma_start(wq_sb[:], wq.ap().rearrange("(kt p) h -> p kt h", p=P))
            nc.sync.dma_start(wk_sb[:], wk.ap().rearrange("(kt p) h -> p kt h", p=P))
            nc.sync.dma_start(wv_sb[:], wv.ap().rearrange("(kt p) h -> p kt h", p=P))

            # wo resident (bf16)
            wo_sb = res.tile([P, NMT, D], DT.bfloat16, tag="wo")
            nc.sync.dma_start(wo_sb[:], wo.ap().rearrange("(mt p) d -> p mt d", p=P))

            # persistent activations
            qt_sb = res.tile([P, NMT, S], DT.bfloat16, tag="qt")   # q REVERSED
            kt_sb = res.tile([P, NMT, S], DT.bfloat16, tag="kt")
            vaug = res.tile([P, NKT, HL, 2 * D_KV], DT.bfloat16, tag="vaug")
            ctxt = res.tile([P, NMT, S], DT.bfloat16, tag="ctxt")
            nc.vector.memset(vaug[:], 1.0)

            # ---------- phase 1a: Q^T, K^T ----------
            for nq in range(NQC):
                q_ps = [ps.tile([P, SC], DT.float32, tag="ps", name=f"qps{nq}_{i}") for i in range(NMT)]
                k_ps = [ps.tile([P, SC], DT.float32, tag="ps", name=f"kps{nq}_{i}") for i in range(NMT)]
                for kd in range(NDT):
                    xt_t = xtp.tile([P, SC], DT.bfloat16, tag="xt")
                    nc.sync.dma_start(
                        xt_t[:], xt[kd * P:(kd + 1) * P, nq * SC:(nq + 1) * SC]
                    )
                    for m in range(NMT):
                        nc.tensor.matmul(
                            q_ps[m][:], wq_sb[:, kd, m * P:(m + 1) * P], xt_t[:],
                            start=(kd == 0), stop=(kd == NDT - 1),
                        )
                        nc.tensor.matmul(
                            k_ps[m][:], wk_sb[:, kd, m * P:(m + 1) * P], xt_t[:],
                            start=(kd == 0), stop=(kd == NDT - 1),
                        )
                for m in range(NMT):
                    # Q^T written q-reversed: dst col (S-1 - q), q ascending
                    dst = qt_sb[:, m, :]
                    rev = bass.AP(
                        tensor=dst.tensor,
                        offset=dst.offset + (S - 1 - nq * SC),
                        ap=[list(dst.ap[0]), [-1, SC]],
                    )
                    nc.vector.tensor_copy(rev, q_ps[m][:])
                    nc.vector.tensor_copy(
                        kt_sb[:, m, nq * SC:(nq + 1) * SC], k_ps[m][:]
                    )

            # ---------- phase 1b: V ----------
            for nq in range(NQC):
                v_ps = [ps.tile([P, SC], DT.float32, tag="ps", name=f"vps{nq}_{i}") for i in range(4)]
                for kd in range(NDT):
                    xt_t = xtp.tile([P, SC], DT.bfloat16, tag="xt")
                    nc.sync.dma_start(
                        xt_t[:], xt[kd * P:(kd + 1) * P, nq * SC:(nq + 1) * SC]
                    )
                    for st in range(4):
                        nc.tensor.matmul(
                            v_ps[st][:], xt_t[:, st * P:(st + 1) * P], wv_sb[:, kd, :],
                            start=(kd == 0), stop=(kd == NDT - 1),
                        )
                for st in range(4):
                    kt_glob = nq * 4 + st
                    nc.vector.tensor_copy(
                        vaug[:, kt_glob, :, 0:D_KV],
                        v_ps[st][:].rearrange("p (h d) -> p h d", d=D_KV),
                    )

            # ---------- phase 2: attention ----------
            for pr in range(HL // 2):
                h0, h1 = 2 * pr, 2 * pr + 1
                u_t = {}
                for hh in (h0, h1):
                    u = upool.tile([P, W_U], DT.bfloat16, tag="u", name=f"u{pr}_{hh}")
                    shear = bass.AP(
                        tensor=ediag_dram.tensor,
                        offset=ediag_dram.offset + hh * NDIAG,
                        ap=[[1, P], [1, W_U]],
                    )
                    nc.sync.dma_start(u[:], shear)
                    u_t[hh] = u
                for qc in range(NQC):
                    jg0 = qc * SC
                    cx0 = ps.tile([P, SC], DT.float32, tag="ps")
                    cx1 = ps.tile([P, SC], DT.float32, tag="ps")
                    for kt in range(NKT):
                        s0 = ps.tile([P, SC], DT.float32, tag="ps")
                        s1 = ps.tile([P, SC], DT.float32, tag="ps")
                        nc.tensor.matmul(
                            s0[:], kt_sb[0:64, pr, kt * P:(kt + 1) * P],
                            qt_sb[0:64, pr, jg0:jg0 + SC],
                            start=True, stop=True, tile_position=(0, 0),
                        )
                        nc.tensor.matmul(
                            s1[:], kt_sb[64:128, pr, kt * P:(kt + 1) * P],
                            qt_sb[64:128, pr, jg0:jg0 + SC],
                            start=True, stop=True, tile_position=(64, 0),
                        )
                        px0 = pexpp.tile([P, SC], DT.bfloat16, tag="pexp")
                        px1 = pexpp.tile([P, SC], DT.bfloat16, tag="pexp")
                        nc.scalar.activation(
                            out=px0[:], in_=s0[:], func=AF.Exp,
                            bias=mask_sb[:, kt:kt + 1], scale=1.0 / math.sqrt(D_KV),
                        )
                        nc.scalar.activation(
                            out=px1[:], in_=s1[:], func=AF.Exp,
                            bias=mask_sb[:, kt:kt + 1], scale=1.0 / math.sqrt(D_KV),
                        )
                        j0 = kt * P + jg0
                        nc.vector.tensor_tensor(
                            px0[:], px0[:], u_t[h0][:, j0:j0 + SC], OP.mult
                        )
                        nc.vector.tensor_tensor(
                            px1[:], px1[:], u_t[h1][:, j0:j0 + SC], OP.mult
                        )
                        nc.tensor.matmul(
                            cx0[:], vaug[:, kt, h0, :], px0[:],
                            start=(kt == 0), stop=(kt == NKT - 1),
                        )
                        nc.tensor.matmul(
                            cx1[:], vaug[:, kt, h1, :], px1[:],
                            start=(kt == 0), stop=(kt == NKT - 1),
                        )
                    for hh, cx in ((h0, cx0), (h1, cx1)):
                        # psum rows 64:128 hold the softmax denominator
                        # (replicated by the ones-block in V_aug)
                        rcp_hi = stage.tile([P, SC], DT.float32, tag="rcp_hi")
                        nc.vector.reciprocal(rcp_hi[64:128, :], cx[64:128, :])
                        rcp_lo = stage.tile([D_KV, SC], DT.float32, tag="rcp_lo")
                        nc.sync.dma_start(rcp_lo[:], rcp_hi[64:128, :])
                        # un-reverse q while normalizing: psum col f -> q = S-1-(jg0+f)
                        base = ctxt[(hh % 2) * 64:(hh % 2) * 64 + 64, pr, :]
                        dst = bass.AP(
                            tensor=base.tensor,
                            offset=base.offset + (S - 1 - jg0),
                            ap=[list(base.ap[0]), [-1, SC]],
                        )
                        nc.vector.tensor_tensor(
                            dst, cx[0:D_KV, :], rcp_lo[:], OP.mult,
                        )

            # ---------- phase 3: output projection ----------
            for st in range(NKT):
                for nd in range(NQC):
                    o_ps = ps.tile([P, SC], DT.float32, tag="ps")
                    for m in range(NMT):
                        nc.tensor.matmul(
                            o_ps[:], ctxt[:, m, st * P:(st + 1) * P],
                            wo_sb[:, m, nd * SC:(nd + 1) * SC],
                            start=(m == 0), stop=(m == NMT - 1),
                        )
                    o_t = outp.tile([P, SC], DT.float32, tag="out")
                    nc.scalar.copy(o_t[:], o_ps[:])
                    nc.sync.dma_start(
                        out[st * P:(st + 1) * P, nd * SC:(nd + 1) * SC], o_t[:]
                    )

    nc.finalize()
    return nc


_NC_CACHE = None


def _get_nc():
    global _NC_CACHE
    if _NC_CACHE is None:
        _NC_CACHE = _build()
    return _NC_CACHE


def _in_maps(hidden_states, attention_mask, Wq, Wk, Wv, Wo, rel_emb):
    oh = _onehot_const()
    import ml_dtypes
    bf16 = ml_dtypes.bfloat16
    maps = []
    for c in range(NCORES):
        b, g = c // 4, c % 4
        hlo, hhi = g * HL, (g + 1) * HL
        maps.append({
            "xt": np.ascontiguousarray(hidden_states[b].T).astype(bf16),
            "wq": np.ascontiguousarray(Wq[:, hlo * D_KV:hhi * D_KV]).astype(bf16),
            "wk": np.ascontiguousarray(Wk[:, hlo * D_KV:hhi * D_KV]).astype(bf16),
            "wv": np.ascontiguousarray(Wv[:, hlo * D_KV:hhi * D_KV]).astype(bf16),
            "wo": np.ascontiguousarray(Wo[hlo * D_KV:hhi * D_KV, :]).astype(bf16),
            "mask": np.ascontiguousarray(attention_mask[b, 0, 0, :]).astype(np.float32),
            "rel": np.ascontiguousarray(rel_emb[:, hlo:hhi]).astype(np.float32),
            "oh": oh,
        })
    return maps


def kernel(hidden_states, attention_mask, Wq, Wk, Wv, Wo, rel_emb, _trace=False,
           _trace_kwargs=None):
    hidden_states = np.asarray(hidden_states, dtype=np.float32)
    attention_mask = np.asarray(attention_mask, dtype=np.float32)
    Wq = np.asarray(Wq, dtype=np.float32)
    Wk = np.asarray(Wk, dtype=np.float32)
    Wv = np.asarray(Wv, dtype=np.float32)
    Wo = np.asarray(Wo, dtype=np.float32)
    rel_emb = np.asarray(rel_emb, dtype=np.float32)

    nc = _get_nc()
    maps = _in_maps(hidden_states, attention_mask, Wq, Wk, Wv, Wo, rel_emb)
    kw = dict(_trace_kwargs or {})
    res = run_bass_kernel_spmd(nc, maps, core_ids=list(range(NCORES)),
                               trace=_trace, **kw)
    kernel.last_results = res
    outp = np.empty((B, S, D), dtype=np.float32)
    for b in range(B):
        acc = np.asarray(res.results[4 * b]["out"], dtype=np.float32).copy()
        for g in range(1, 4):
            acc += np.asarray(res.results[4 * b + g]["out"], dtype=np.float32)
        outp[b] = acc
    return outp
